# revision 12
# baseline (speedup 1.0000x reference)
"""GRU cell kernel for Trainium2, data-parallel across 8 NeuronCores.

Reference computation (per batch row):
    concat = [h_prev, x]                       # [B, 2048]
    z = sigmoid(concat @ W_z.T + b_z)          # [B, 1024]
    r = sigmoid(concat @ W_r.T + b_r)
    h_tilde = tanh([r*h_prev, x] @ W_h.T + b_h)
    h = (1-z)*h_prev + z*h_tilde

Sharding: batch dim (8192) split 1024/core; weights replicated.
Layout on device is feature-major ([feature, batch]) so the batch is the
matmul moving dimension (N=512 per PSUM bank) and the hidden units are the
PSUM partition dim. Host transposes in/out.

mm_dtype selects the matmul precision:
  fp8  — e4m3 operands with DoubleRow perf mode (2 MACs/PE/cycle).
         Weights are pre-scaled by 2^13 so the uniform(-1/sqrt(2048))
         init lands in e4m3's normal range (max 181 < 240); the 2^-13
         is folded into the activation's input scale. Elementwise path
         (r*h_prev, final combine) runs in bf16.
  f32r — TF32 PE mode, fp32 storage (rel err ~1e-4)
  bf16 — bf16 operands (weights/acts), fp32 h_prev kept for elementwise
  f32  — true fp32 matmuls (4x slower on PE)
"""

import numpy as np

import concourse.bacc as bacc
import concourse.bass as bass
import concourse.mybir as mybir
import concourse.tile as tile
from concourse import bass_utils

P = 128
B = 8192
I = 1024
H = 1024
K = I + H            # 2048 contraction
NCORES = 8
BS = B // NCORES     # 1024 batch rows per core
MT = H // P          # 8 m-tiles (hidden units)
KT = K // P          # 16 k-chunks of 128
KT2 = KT // 2        # 8 DoubleRow chunks of 256
NFREE = 512          # matmul moving free dim (one PSUM bank of fp32)
NT = BS // NFREE     # 2 n-tiles per core

WSCALE = 10860.0     # fp8 weight pre-scale: maps max|W|=1/sqrt(2048) to ~240

F32 = mybir.dt.float32
F32R = mybir.dt.float32r
BF16 = mybir.dt.bfloat16
F8 = mybir.dt.float8e4

AF = mybir.ActivationFunctionType
DR = mybir.MatmulPerfMode.DoubleRow


def build_kernel(mm_dtype: str = "fp8"):
    """Build the per-core Bass kernel. Returns compiled nc."""
    fp8 = mm_dtype == "fp8"
    bf16 = mm_dtype == "bf16"
    mdt = {"fp8": F8, "f32r": F32R, "f32": F32, "bf16": BF16}[mm_dtype]
    # dtype of gate intermediates (z, r_tmp, ht) and the output store
    gdt = BF16 if fp8 else F32
    act_scale = 1.0 / WSCALE if fp8 else 1.0
    nc = bacc.Bacc("TRN2", target_bir_lowering=False, debug=False)

    # DRAM I/O (per-core shapes). Matmul-feeding tensors carry the matmul
    # dtype (f32r is the same bits as f32 on the host side).
    xT = nc.dram_tensor("xT", [I, BS], mdt, kind="ExternalInput").ap()
    hT = nc.dram_tensor("hT", [H, BS], mdt, kind="ExternalInput").ap()
    if fp8 or bf16:  # separate elementwise-precision copy of h_prev
        ew_dt = BF16 if fp8 else F32
        hTe = nc.dram_tensor("hTe", [H, BS], ew_dt, kind="ExternalInput").ap()
    # weights as [MT, P, KT, P]: per m-tile, partition-major, k-chunked
    Wz = nc.dram_tensor("Wz", [MT, P, KT, P], mdt, kind="ExternalInput").ap()
    Wr = nc.dram_tensor("Wr", [MT, P, KT, P], mdt, kind="ExternalInput").ap()
    Wh = nc.dram_tensor("Wh", [MT, P, KT, P], mdt, kind="ExternalInput").ap()
    bz = nc.dram_tensor("bz", [P, MT], F32, kind="ExternalInput").ap()
    br = nc.dram_tensor("br", [P, MT], F32, kind="ExternalInput").ap()
    bh = nc.dram_tensor("bh", [P, MT], F32, kind="ExternalInput").ap()
    out = nc.dram_tensor("out", [H, BS], gdt, kind="ExternalOutput").ap()

    def ew(ap):
        """fp32 view of an f32r AP for elementwise use (same bits)."""
        return ap.bitcast(F32) if mdt == F32R else ap

    with tile.TileContext(nc) as tc:
        with (
            tc.tile_pool(name="acts", bufs=1) as acts,
            tc.tile_pool(name="gates", bufs=1) as gates,
            tc.tile_pool(name="wpool", bufs=5) as wpool,
            tc.tile_pool(name="opool", bufs=6) as opool,
            tc.tile_pool(name="ppool", bufs=8, space="PSUM") as ppool,
        ):
            # Biases first — they gate the first sigmoid (psum recycling).
            # Scalar HWDGE ring so they don't sit behind the act loads.
            bz_sb = acts.tile([P, MT], F32)
            br_sb = acts.tile([P, MT], F32)
            bh_sb = acts.tile([P, MT], F32)
            nc.scalar.dma_start(br_sb[:], br)
            nc.scalar.dma_start(bz_sb[:], bz)
            nc.scalar.dma_start(bh_sb[:], bh)

            # First two weight tiles go at the HEAD of the sync HWDGE ring:
            # within a ring DMAs drain FIFO, so they get full SDMA bandwidth
            # before the act loads start, instead of a round-robin share.
            # (The SWDGE queue used for the remaining tiles takes ~6us to
            # spin up anyway.)
            w_pre = {("r", i): wpool.tile([P, KT, P], mdt, tag="w",
                                          name=f"wf{i}")
                     for i in range(6)}
            if fp8:
                # Wr0/Wr1 at the head of the scalar ring, in parallel with
                # the act bytes on the sync ring (first MM needs Wr0 +
                # hT[0:4]-n0 — put them on different rings).
                nc.scalar.dma_start(w_pre[("r", 0)][:], Wr[0])
                nc.scalar.dma_start(w_pre[("r", 1)][:], Wr[1])
            else:
                nc.sync.dma_start(w_pre[("r", 0)][:], Wr[0])
                nc.scalar.dma_start(w_pre[("r", 1)][:], Wr[1])
                nc.scalar.dma_start(w_pre[("r", 3)][:], Wr[3])

            # Pre-warm the ACT sigmoid/tanh table set during the DMA fill so
            # the first real sigmoid doesn't pay the ~2.7us ACT_TABLE_LOAD.
            # Reads its own uninitialized tile — no DMA dependency, result
            # discarded — so it cannot stall the scalar ring's weight DMAs.
            if fp8:
                # Wr3's descriptor must be queued before the warm
                # activation: the table load blocks the scalar engine's
                # in-order FIFO for ~2.7us.
                nc.scalar.dma_start(w_pre[("r", 3)][:], Wr[3])
            warm = acts.tile([P, 1], F32)
            nc.scalar.activation(warm[:], warm[:], AF.Sigmoid)

            if fp8:
                # HAM pre-warm: junk matmuls (uninitialized operands, result
                # never read) keep the PE busy during the DMA fill so the
                # activity monitor lifts the clock gate to 8/8 before the
                # real matmul stream starts. ~6 x 512-col cold matmuls span
                # the ~3.4us activity window; they finish right as the
                # first real matmul's inputs land.
                warm_mm = acts.tile([P, NFREE], mdt, name="warm_mm")
                nc.vector.memset(warm_mm[:], 0)
                warm_ps = ppool.tile([P, NFREE], F32, tag="ps",
                                     name="warm_ps")
                for _ in range(6):
                    nc.tensor.matmul(warm_ps, warm_mm[:, 0:P], warm_mm[:],
                                     start=True, stop=True)

            # Persistent activations, feature-major: [p, ko, batch]
            xT_sb = acts.tile([P, I // P, BS], mdt)
            hT_sb = acts.tile([P, H // P, BS], mdt)
            hTe_sb = (acts.tile([P, H // P, BS], BF16 if fp8 else F32,
                                name="hTe_sb")
                      if (fp8 or bf16) else None)
            # Load per (tensor, batch-half), n=0 halves first, so the first
            # PSUM groups (n=0) are gated on half the act bytes. One DMA per
            # half: each dma_start costs ~600ns of descriptor-gen serialized
            # on the sync sequencer, so many small chunk loads would delay
            # the bytes the first PSUM group needs. Weight DMAs ride the
            # idle GpSimd SWDGE queue so they don't serialize with act
            # loads or compute issue.
            xT_r = xT.rearrange("(ko p) b -> p ko b", p=P)
            hT_r = hT.rearrange("(ko p) b -> p ko b", p=P)
            hTe_r = (hTe.rearrange("(ko p) b -> p ko b", p=P)
                     if (fp8 or bf16) else None)
            # Interleave the next R-gate weight tiles into the sync FIFO so
            # they drain right AFTER the bytes the first PSUM groups need,
            # instead of contending with them from the gpsimd ring.
            if fp8:
                # hT-n0 split in two so the ramp's first matmuls gate on
                # 256KB, not 512KB.
                n0 = slice(0, NFREE)
                nc.sync.dma_start(hT_sb[:, 0:4, n0], hT_r[:, 0:4, n0])
                nc.sync.dma_start(hT_sb[:, 4:8, n0], hT_r[:, 4:8, n0])
                nc.sync.dma_start(w_pre[("r", 2)][:], Wr[2])
                nc.sync.dma_start(xT_sb[:, :, n0], xT_r[:, :, n0])
                n1 = slice(NFREE, BS)
                nc.sync.dma_start(hT_sb[:, :, n1], hT_r[:, :, n1])
                nc.sync.dma_start(w_pre[("r", 4)][:], Wr[4])
                nc.sync.dma_start(xT_sb[:, :, n1], xT_r[:, :, n1])
                for half in range(2):
                    ks = slice(half * 4, (half + 1) * 4)
                    nc.sync.dma_start(hTe_sb[:, ks, :], hTe_r[:, ks, :])
                nc.sync.dma_start(w_pre[("r", 5)][:], Wr[5])
            else:
                for n in range(NT):
                    ns = slice(n * NFREE, (n + 1) * NFREE)
                    nc.sync.dma_start(hT_sb[:, :, ns], hT_r[:, :, ns])
                    if n == 0:
                        # w2 drains before xT-n0: the interleaved ramp
                        # consumes (w0..w3, hT-n0) first and must not
                        # head-of-line block.
                        nc.sync.dma_start(w_pre[("r", 2)][:], Wr[2])
                    nc.sync.dma_start(xT_sb[:, :, ns], xT_r[:, :, ns])
                    if n == 1:
                        nc.sync.dma_start(w_pre[("r", 4)][:], Wr[4])
                if bf16:
                    for half in range(2):
                        ks = slice(half * 4, (half + 1) * 4)
                        nc.sync.dma_start(hTe_sb[:, ks, :], hTe_r[:, ks, :])
                nc.sync.dma_start(w_pre[("r", 5)][:], Wr[5])

            # Gate results, feature-major
            z_sb = gates.tile([P, MT, BS], gdt)
            rh_sb = gates.tile([P, MT, BS], mdt)
            # (1-z)*h_prev, precomputed in the z phase (DVE slack) so the
            # h-phase critical chain after the last matmul is only
            # tanh -> mul -> add -> store.
            a_sb = (gates.tile([P, MT, BS], gdt, name="a_sb")
                    if fp8 else None)

            def hprev_ew(mt, ns):
                """elementwise-precision h_prev slice."""
                if fp8 or bf16:
                    return hTe_sb[:, mt, ns]
                return ew(hT_sb[:, mt, ns])

            def issue_mms(ps, w_sb, base, width, stage):
                """Full contraction into psum ps for one (m-tile, n-slice)."""
                if fp8:
                    for k in range(KT2):
                        if 2 * k < H // P:
                            src = rh_sb if stage == "h" else hT_sb
                            rhs = src[:, 2 * k:2 * k + 2, base:base + width]
                        else:
                            ko = 2 * k - H // P
                            rhs = xT_sb[:, ko:ko + 2, base:base + width]
                        nc.tensor.matmul(
                            ps, w_sb[:, 2 * k:2 * k + 2, :], rhs,
                            start=(k == 0), stop=(k == KT2 - 1),
                            perf_mode=DR)
                else:
                    for k in range(KT):
                        if k < H // P:
                            src = rh_sb if stage == "h" else hT_sb
                            rhs = src[:, k, base:base + width]
                        else:
                            rhs = xT_sb[:, k - H // P, base:base + width]
                        nc.tensor.matmul(
                            ps, w_sb[:, k, :], rhs,
                            start=(k == 0), stop=(k == KT - 1))

            def gate(stage, Wd, b_sb):
                if stage == "r":
                    # Interleaved ramp: open 4 PSUM groups (mt0-3, n=0),
                    # k-outer across them, so the PE runs matmuls on the
                    # already-arrived h_prev half while the x half of the
                    # batch (and later weights) are still in flight.
                    NG = 4
                    ws = [w_pre[("r", g)] for g in range(NG)]
                    pss = [ppool.tile([P, NFREE], F32, tag="ps",
                                      name=f"psri{g}") for g in range(NG)]
                    nk = KT2 if fp8 else KT
                    for k in range(nk):
                        for g in range(NG):
                            if fp8:
                                if 2 * k < H // P:
                                    rhs = hT_sb[:, 2 * k:2 * k + 2, 0:NFREE]
                                else:
                                    ko = 2 * k - H // P
                                    rhs = xT_sb[:, ko:ko + 2, 0:NFREE]
                                nc.tensor.matmul(
                                    pss[g], ws[g][:, 2 * k:2 * k + 2, :],
                                    rhs, start=(k == 0), stop=(k == nk - 1),
                                    perf_mode=DR)
                            else:
                                if k < H // P:
                                    rhs = hT_sb[:, k, 0:NFREE]
                                else:
                                    rhs = xT_sb[:, k - H // P, 0:NFREE]
                                nc.tensor.matmul(
                                    pss[g], ws[g][:, k, :], rhs,
                                    start=(k == 0), stop=(k == nk - 1))
                    for g in range(NG):
                        ns0 = slice(0, NFREE)
                        r_tmp = opool.tile([P, NFREE], gdt, tag="rt",
                                           name=f"rti{g}")
                        nc.scalar.activation(
                            r_tmp, pss[g], AF.Sigmoid, bias=b_sb[:, g:g + 1],
                            scale=act_scale)
                        nc.vector.tensor_mul(
                            rh_sb[:, g, ns0], r_tmp, hprev_ew(g, ns0))
                    plan = ([(mt, 1) for mt in range(NG)]
                            + [(mt, n) for mt in range(NG, MT)
                               for n in range(NT)])
                else:
                    plan = [(mt, n) for mt in range(MT) for n in range(NT)]
                for mt, n in plan:
                    w_sb = w_pre.get((stage, mt))
                    if w_sb is None:
                        w_sb = wpool.tile([P, KT, P], mdt, tag="w")
                        nc.gpsimd.dma_start(w_sb[:], Wd[mt])
                        w_pre[(stage, mt)] = w_sb
                    # Split the very last group so its activation+combine
                    # +store chain pipelines instead of sitting fully
                    # exposed after the final matmul.
                    last = stage == "h" and mt == MT - 1 and n == NT - 1
                    widths = ([256, 128, 128] if last and fp8
                              else [256, 256] if last
                              else [NFREE])
                    base = n * NFREE
                    for s, width in enumerate(widths):
                        ps = ppool.tile([P, width], F32, tag="ps",
                                        name=f"ps{mt}_{n}_{s}")
                        issue_mms(ps, w_sb, base, width, stage)
                        ns = slice(base, base + width)
                        base += width
                        bias = b_sb[:, mt:mt + 1]
                        if stage == "r":
                            # r -> rh = r * h_prev, in matmul dtype
                            r_tmp = opool.tile([P, width], gdt, tag="rt")
                            nc.scalar.activation(
                                r_tmp, ps, AF.Sigmoid, bias=bias,
                                scale=act_scale)
                            nc.vector.tensor_mul(
                                rh_sb[:, mt, ns], r_tmp, hprev_ew(mt, ns))
                        elif stage == "z":
                            nc.scalar.activation(
                                z_sb[:, mt, ns], ps, AF.Sigmoid, bias=bias,
                                scale=act_scale)
                            if fp8:
                                zt = opool.tile([P, width], gdt, tag="zt")
                                nc.vector.tensor_mul(
                                    zt, z_sb[:, mt, ns], hprev_ew(mt, ns))
                                nc.vector.tensor_sub(
                                    a_sb[:, mt, ns], hprev_ew(mt, ns), zt)
                        elif fp8:  # h = z*tanh(pre) + (h_prev - z*h_prev)
                            ht = opool.tile([P, width], gdt, tag="ht",
                                            name=f"ht{mt}_{n}_{s}")
                            nc.scalar.activation(
                                ht, ps, AF.Tanh, bias=bias, scale=act_scale)
                            nc.vector.tensor_mul(ht, ht, z_sb[:, mt, ns])
                            nc.vector.tensor_add(ht, ht, a_sb[:, mt, ns])
                            nc.sync.dma_start(
                                out[mt * P:(mt + 1) * P, ns], ht)
                        else:  # h = h_prev + z*(tanh(pre) - h_prev)
                            ht = opool.tile([P, width], gdt, tag="ht",
                                            name=f"ht{mt}_{n}_{s}")
                            nc.scalar.activation(
                                ht, ps, AF.Tanh, bias=bias, scale=act_scale)
                            nc.vector.tensor_sub(ht, ht, hprev_ew(mt, ns))
                            nc.vector.tensor_mul(ht, ht, z_sb[:, mt, ns])
                            nc.vector.tensor_add(ht, ht, hprev_ew(mt, ns))
                            nc.sync.dma_start(
                                out[mt * P:(mt + 1) * P, ns], ht)

            gate("r", Wr, br_sb)
            gate("z", Wz, bz_sb)
            gate("h", Wh, bh_sb)

    nc.compile()
    return nc


def _prep_inputs(x, h_prev, W_z, b_z, W_r, b_r, W_h, b_h,
                 mm_dtype="fp8"):
    """Host-side relayout: feature-major activations, m-tiled weights."""
    import ml_dtypes
    fp8 = mm_dtype == "fp8"
    bf16 = mm_dtype == "bf16"
    np_dtype = {"fp8": ml_dtypes.float8_e4m3, "bf16": ml_dtypes.bfloat16,
                "f32r": np.float32, "f32": np.float32}[mm_dtype]
    wscale = WSCALE if fp8 else 1.0

    def prep_w(W):
        # want w[mt, p, ko, m] = W[mt*128+m, ko*128+p]
        W4 = W.reshape(MT, P, KT, P)          # [mt, m, ko, p]
        Wt = np.ascontiguousarray(W4.transpose(0, 3, 2, 1))  # [mt,p,ko,m]
        if fp8:
            Wt = np.clip(Wt * wscale, -240.0, 240.0)
        return Wt.astype(np_dtype)

    def prep_b(b):
        return np.ascontiguousarray(b.reshape(MT, P).T)

    xT = np.ascontiguousarray(x.T)                        # [I, B] f32
    hTf = np.ascontiguousarray(h_prev.T)                  # [H, B] f32
    x8 = xT.astype(np_dtype)
    h8 = hTf.astype(np_dtype)
    if fp8:
        hTe = hTf.astype(ml_dtypes.bfloat16)
    else:
        hTe = hTf
    shared = {
        "Wz": prep_w(W_z), "Wr": prep_w(W_r), "Wh": prep_w(W_h),
        "bz": prep_b(b_z), "br": prep_b(b_r), "bh": prep_b(b_h),
    }
    in_maps = []
    for c in range(NCORES):
        bs = slice(c * BS, (c + 1) * BS)
        m = dict(shared)
        m["xT"] = np.ascontiguousarray(x8[:, bs])
        m["hT"] = np.ascontiguousarray(h8[:, bs])
        if fp8 or bf16:
            m["hTe"] = np.ascontiguousarray(hTe[:, bs])
        in_maps.append(m)
    return in_maps


def run(inputs, mm_dtype="fp8", trace=False, **run_kwargs):
    """Compile + run on 8 cores. Returns (output [B,H] f32, BassKernelResults)."""
    nc = build_kernel(mm_dtype)
    in_maps = _prep_inputs(**inputs, mm_dtype=mm_dtype)
    res = bass_utils.run_bass_kernel_spmd(
        nc, in_maps, core_ids=list(range(NCORES)), trace=trace, **run_kwargs)
    outT = np.concatenate(
        [np.asarray(res.results[c]["out"]) for c in range(NCORES)],
        axis=1)  # [H, B]
    return np.ascontiguousarray(outT.T).astype(np.float32), res


def kernel(**inputs) -> np.ndarray:
    import time as _time
    try:
        out, _ = run(inputs)
    except Exception:
        # The axon-tunneled device occasionally reports a transient
        # "unrecoverable" state right after a crashed session; a fresh
        # attempt after a short pause recovers.
        _time.sleep(15)
        out, _ = run(inputs)
    return out
